# revision 1
# baseline (speedup 1.0000x reference)
"""Trainium2 Bass kernel for nn_CandidateSubgraphTFModel (gnn_message_passing).

Strategy (8 NeuronCores, SPMD, no collectives):
  - Data-parallel over batch B=32 -> 4 rows/core for the TCN/encode path.
  - GCN (candidate_embed) replicated on every core.
  - All matmuls in fp32r (fp32 rounded to 11 mantissa bits, 1 cyc/row on PE).
  - Feature-major activation layout [D(part), tokens(free)]; LN stats via
    ones-matvec on PE + K=1 broadcast matmuls; degree norm folded
    algebraically (d_j into X-tilde, d_i applied after the gcn_w matmul).

kernel(**inputs) takes FULL inputs (as in reference.setup_inputs()) and
returns the FULL [32, 1024, 2048] logits.
"""
import sys
sys.path.insert(0, '/opt/trn_rl_repo')
import numpy as np

import concourse.bass as bass
import concourse.bacc as bacc
import concourse.tile as tile
from concourse import mybir
from concourse.masks import make_identity

f32 = mybir.dt.float32
f32r = mybir.dt.float32r
i32 = mybir.dt.int32
AF = mybir.ActivationFunctionType
OP = mybir.AluOpType

LN_EPS = 1e-5


class Cfg:
    def __init__(self, V=50000, D=512, B=32, S=1024, N=2048, K=3, n_cores=8):
        self.V, self.D, self.B, self.S, self.N, self.K = V, D, B, S, N, K
        self.n_cores = n_cores
        self.B_loc = B // n_cores
        self.DC = D // 128          # feature chunks
        self.ST = min(512, S)       # supertile (tokens)
        self.NTT = self.ST // 128   # token-tiles per supertile
        self.NST = S // self.ST     # supertiles per row
        self.NJC = N // 128         # GCN j chunks
        self.SW = min(512, N)       # slice width over N
        self.NSL = N // self.SW     # slices over N
        self.TOK = self.B_loc * S   # tokens per core


def build_program(cfg, reps=1, has_c0=False):
    c = cfg
    nc = bacc.Bacc("TRN2", target_bir_lowering=False, debug=False)

    x_in = nc.dram_tensor("x_in_loc", [c.TOK], i32, kind="ExternalInput").ap()
    mask = nc.dram_tensor("mask_loc", [c.TOK], f32, kind="ExternalInput").ap()
    sub_nodes = nc.dram_tensor("sub_nodes", [c.N], i32, kind="ExternalInput").ap()
    A_T = nc.dram_tensor("A_subT", [c.N, c.N], f32, kind="ExternalInput").ap()
    embed = nc.dram_tensor("embed", [c.V, c.D], f32, kind="ExternalInput").ap()
    c1w = nc.dram_tensor("conv1_w", [c.K, c.D, c.D], f32, kind="ExternalInput").ap()
    c1b = nc.dram_tensor("conv1_b", [c.D], f32, kind="ExternalInput").ap()
    c2w = nc.dram_tensor("conv2_w", [c.K, c.D, c.D], f32, kind="ExternalInput").ap()
    c2b = nc.dram_tensor("conv2_b", [c.D], f32, kind="ExternalInput").ap()
    ln_g = nc.dram_tensor("ln_g", [c.D], f32, kind="ExternalInput").ap()
    ln_b = nc.dram_tensor("ln_b", [c.D], f32, kind="ExternalInput").ap()
    gwT = nc.dram_tensor("gcn_wT", [c.D, c.D], f32, kind="ExternalInput").ap()
    gb = nc.dram_tensor("gcn_b", [c.D], f32, kind="ExternalInput").ap()
    hwT = nc.dram_tensor("hproj_wT", [c.D, c.D], f32, kind="ExternalInput").ap()
    out = nc.dram_tensor("logits_loc", [c.TOK, c.N], f32, kind="ExternalOutput").ap()

    with tile.TileContext(nc) as tc:
        for _ in range(reps):
            build_body(tc, c, x_in, mask, sub_nodes, A_T, embed, c1w, c1b,
                       c2w, c2b, ln_g, ln_b, gwT, gb, hwT, out, has_c0)
    nc.compile()
    return nc


def build_body(tc, c, x_in, mask, sub_nodes, A_T, embed, c1w, c1b, c2w, c2b,
               ln_g, ln_b, gwT, gb, hwT, out, has_c0=False):
    nc = tc.nc
    DC, K, ST, NTT, NST, NJC, SW, NSL = (c.DC, c.K, c.ST, c.NTT, c.NST,
                                         c.NJC, c.SW, c.NSL)

    # ---------------- pools (stack order matters) ----------------
    const = tc.alloc_tile_pool(name="const", bufs=1)
    htp = tc.alloc_tile_pool(name="htp", bufs=1)
    wp = tc.alloc_tile_pool(name="wp", bufs=1)

    # ---------------- phase 0: constants + weights ----------------
    ident = const.tile([128, 128], f32)
    make_identity(nc, ident[:])
    ones_f = const.tile([128, 1], f32)
    nc.vector.memset(ones_f[:], 1.0)
    onesr_f = const.tile([1, 128], f32)
    nc.vector.memset(onesr_f[:], 1.0)
    ones_col = const.tile([128, 1], f32r)
    nc.vector.tensor_copy(out=ones_col[:], in_=ones_f[:])
    ones_row = const.tile([1, 128], f32r)
    nc.vector.tensor_copy(out=ones_row[:], in_=onesr_f[:])
    zpad = const.tile([128, 2], f32)
    nc.vector.memset(zpad[:], 0.0)

    def load_cols(dram_vec, name, dtype=f32):
        cols = []
        for dc in range(DC):
            t = const.tile([128, 1], f32, name=f"{name}_{dc}")
            nc.sync.dma_start(out=t[:], in_=dram_vec[dc*128:(dc+1)*128, None])
            if dtype is f32r:
                tr = const.tile([128, 1], f32r, name=f"{name}r_{dc}")
                nc.vector.tensor_copy(out=tr[:], in_=t[:])
                cols.append(tr)
            else:
                cols.append(t)
        return cols

    b1_col = load_cols(c1b, "b1")
    b2_col = load_cols(c2b, "b2")
    g_col = load_cols(ln_g, "g")
    lnb_colr = load_cols(ln_b, "lnb", f32r) if has_c0 else None
    gb_col = load_cols(gb, "gb")

    # conv weights -> f32r tiles [128(din), D(dout)] per (k, din_chunk)
    stage = tc.alloc_tile_pool(name="stage", bufs=3)
    w1r, w2r = [], []
    for (wsrc, wdst) in ((c1w, w1r), (c2w, w2r)):
        for k in range(K):
            for dc in range(DC):
                st_ = stage.tile([128, c.D], f32, name="wstg")
                nc.sync.dma_start(out=st_[:], in_=wsrc[k, dc*128:(dc+1)*128, :])
                wr = wp.tile([128, c.D], f32r, name=f"w_{len(wdst)}_{id(wdst)%97}")
                nc.vector.tensor_copy(out=wr[:], in_=st_[:])
                wdst.append(wr)

    # hproj: wg = g (x) hwT rows (f32r); c0 = ln_b @ hwT
    pp0 = tc.alloc_tile_pool(name="pp0", bufs=1, space="PSUM")
    wgr, hwr_tmp = [], []
    for dc in range(DC):
        st_ = stage.tile([128, c.D], f32, name="wstg2")
        nc.sync.dma_start(out=st_[:], in_=hwT[dc*128:(dc+1)*128, :])
        wg = wp.tile([128, c.D], f32r, name=f"wg_{dc}")
        nc.vector.tensor_scalar_mul(wg[:], st_[:], g_col[dc][:])
        wgr.append(wg)
        hr = stage.tile([128, c.D], f32r, name="hwr_tmp", bufs=2)
        nc.scalar.copy(out=hr[:], in_=st_[:])
        hwr_tmp.append(hr)
    # u_neg[e] = -sum_d Wg[d, e]  (for folding LN (z-mu)*rstd past hproj)
    u_ps = pp0.tile([1, c.D], f32, tag="c0ps")
    for dc in range(DC):
        nc.tensor.matmul(out=u_ps[:], lhsT=ones_col[:], rhs=wgr[dc][:],
                         start=(dc == 0), stop=(dc == DC - 1))
    u_row = const.tile([1, c.D], f32)
    nc.scalar.mul(u_row[:], u_ps[:], -1.0)
    u_neg_col = []
    for ec in range(DC):
        tp_ps = pp0.tile([128, 1], f32, name="ut", tag="c0t")
        nc.tensor.transpose(out=tp_ps[:], in_=u_row[0:1, ec*128:(ec+1)*128],
                            identity=ident[0:1, 0:1])
        uc = const.tile([128, 1], f32, name=f"unc_{ec}")
        nc.vector.tensor_copy(out=uc[:], in_=tp_ps[:])
        u_neg_col.append(uc)
    c0_col = None
    if has_c0:
        c0_ps = pp0.tile([1, c.D], f32, tag="c0ps")
        for dc in range(DC):
            nc.tensor.matmul(out=c0_ps[:], lhsT=lnb_colr[dc][:],
                             rhs=hwr_tmp[dc][:],
                             start=(dc == 0), stop=(dc == DC - 1))
        c0_row = const.tile([1, c.D], f32)
        nc.vector.tensor_copy(out=c0_row[:], in_=c0_ps[:])
        c0_col = []
        for ec in range(DC):
            tp_ps = pp0.tile([128, 1], f32, name="c0t", tag="c0t")
            nc.tensor.transpose(out=tp_ps[:], in_=c0_row[0:1, ec*128:(ec+1)*128],
                                identity=ident[0:1, 0:1])
            cc = const.tile([128, 1], f32, name=f"c0c_{ec}")
            nc.vector.tensor_copy(out=cc[:], in_=tp_ps[:])
            c0_col.append(cc)

    # gcn weights f32r
    gwr = []
    for dc in range(DC):
        st_ = stage.tile([128, c.D], f32, name="wstg3")
        nc.sync.dma_start(out=st_[:], in_=gwT[dc*128:(dc+1)*128, :])
        gr = wp.tile([128, c.D], f32r, name=f"gw_{dc}")
        nc.vector.tensor_copy(out=gr[:], in_=st_[:])
        gwr.append(gr)
    pp0.release()
    stage.release()

    # HT[ec] : [128, N] f32r  (candidate embeddings, feature-major, resident)
    HT = [htp.tile([128, c.N], f32r, name=f"HT_{ec}") for ec in range(DC)]

    # ================= GCN phase =================
    gp = tc.alloc_tile_pool(name="gp", bufs=1)

    # gather X = embed[sub_nodes] -> rounded resident Xr tiles (f32r)
    Xr = []
    for jc in range(NJC):
        sidx = gp.tile([128, 1], i32, name=f"sidx_{jc}")
        nc.sync.dma_start(out=sidx[:], in_=sub_nodes[jc*128:(jc+1)*128, None])
        xstg = gp.tile([128, c.D], f32, name="xstg", bufs=3)
        nc.gpsimd.indirect_dma_start(
            out=xstg[:], out_offset=None, in_=embed[:],
            in_offset=bass.IndirectOffsetOnAxis(ap=sidx[:, :1], axis=0))
        xt = gp.tile([128, c.D], f32r, name=f"Xg_{jc}")
        nc.vector.tensor_copy(out=xt[:], in_=xstg[:])
        Xr.append(xt)

    # pass 1: row sums s[i] = sum_j A_hatT[j, i]  (diag folded as +1 later)
    pg1 = tc.alloc_tile_pool(name="pg1", bufs=1, space="PSUM")
    s_ps = pg1.tile([1, c.N], f32, tag="gs")
    for jc in range(NJC):
        for sl in range(NSL):
            a1 = gp.tile([128, SW], f32, name="a2", bufs=5)
            nc.sync.dma_start(out=a1[:], in_=A_T[jc*128:(jc+1)*128,
                                               sl*SW:(sl+1)*SW])
            ar = gp.tile([128, SW], f32r, name="ar2", bufs=5)
            nc.scalar.copy(out=ar[:], in_=a1[:, :])
            nc.tensor.matmul(out=s_ps[0:1, sl*SW:(sl+1)*SW], lhsT=ones_col[:],
                             rhs=ar[:], start=(jc == 0), stop=(jc == NJC - 1))

    # d = rsqrt(s + 1)
    s1 = gp.tile([1, c.N], f32, name="s1")
    nc.scalar.add(s1[:], s_ps[:], 1.0)
    nc.vector.reciprocal(s1[:], s1[:])
    d_rt = gp.tile([1, c.N], f32r, name="d_rt")
    nc.scalar.sqrt(d_rt[:], s1[:])
    d_row = d_rt[:]

    # d_col[jc] [128,1] f32r: fp32 PE transpose of 1/(s+1) then sqrt
    d_col = []
    for jc in range(NJC):
        dt_ps = pg1.tile([128, 1], f32, name="dtp", tag="gt")
        nc.tensor.transpose(out=dt_ps[:], in_=s1[0:1, jc*128:(jc+1)*128],
                            identity=ident[0:1, 0:1])
        dcl = gp.tile([128, 1], f32r, name=f"dcol_{jc}")
        nc.scalar.sqrt(dcl[:], dt_ps[:])
        d_col.append(dcl)

    # d broadcast [128, SW] per slice (K=1 matmul), copied to SBUF
    d_bc = []
    for sl in range(NSL):
        db_ps = pg1.tile([128, SW], f32, name="dbp", tag="gb")
        nc.tensor.matmul(out=db_ps[:], lhsT=ones_row[:],
                         rhs=d_row[0:1, sl*SW:(sl+1)*SW], start=True, stop=True)
        db = gp.tile([128, SW], f32, name=f"dbc_{sl}")
        nc.vector.tensor_copy(out=db[:], in_=db_ps[:])
        d_bc.append(db)
    pg1.release()

    # pass 2: per i-slice: M1T = sum_j Xr[j,:].T-blocks @ (d_j * A_hatT[j, islice])
    #         then HT[:, islice] = relu(d_i * (gw @ M1T) + gcn_b)
    pg2 = tc.alloc_tile_pool(name="pg2", bufs=1, space="PSUM")
    for sl in range(NSL):
        m1_ps = [pg2.tile([128, SW], f32, name=f"m1p_{dc}", tag=f"gm{dc}")
                 for dc in range(DC)]
        for jc in range(NJC):
            a2 = gp.tile([128, SW], f32, name="a2", bufs=5)
            nc.sync.dma_start(out=a2[:], in_=A_T[jc*128:(jc+1)*128,
                                               sl*SW:(sl+1)*SW])
            # diagonal block: A_hat = A_sub + I  (add I before the d_j scale)
            if jc * 128 >= sl * SW and (jc + 1) * 128 <= (sl + 1) * SW:
                off = jc * 128 - sl * SW
                nc.vector.tensor_tensor(
                    out=a2[:, off:off+128], in0=a2[:, off:off+128],
                    in1=ident[:], op=OP.add)
            # rounding copy doubles as the d_j scale
            ar2 = gp.tile([128, SW], f32r, name="ar2", bufs=5)
            nc.vector.tensor_scalar_mul(ar2[:], a2[:],
                                        d_col[jc][:].bitcast(f32))
            for dc in range(DC):
                nc.tensor.matmul(out=m1_ps[dc][:],
                                 lhsT=Xr[jc][:, dc*128:(dc+1)*128],
                                 rhs=ar2[:], start=(jc == 0),
                                 stop=(jc == NJC - 1))
        m1 = []
        for dc in range(DC):
            m1s = gp.tile([128, SW], f32r, name=f"m1s_{dc}", bufs=2)
            nc.scalar.copy(out=m1s[:], in_=m1_ps[dc][:])
            m1.append(m1s)
        for ec in range(DC):
            h2_ps = pg2.tile([128, SW], f32, name=f"h2p_{ec}", tag=f"gh{ec}")
            for dc in range(DC):
                nc.tensor.matmul(out=h2_ps[:],
                                 lhsT=gwr[dc][:, ec*128:(ec+1)*128],
                                 rhs=m1[dc][:], start=(dc == 0),
                                 stop=(dc == DC - 1))
            hd = gp.tile([128, SW], f32, name="hd", bufs=2)
            nc.vector.tensor_mul(hd[:], h2_ps[:], d_bc[sl][:])
            nc.scalar.activation(out=HT[ec][:, sl*SW:(sl+1)*SW], in_=hd[:],
                                 func=AF.Relu, bias=gb_col[ec][:])
    pg2.release()
    gp.release()

    # ================= encode + logits =================
    ep = tc.alloc_tile_pool(name="ep", bufs=1)
    pe = tc.alloc_tile_pool(name="pe", bufs=1, space="PSUM")

    SP = c.S + 2  # row buffer width (2 zero pad cols at left)
    for row in range(c.B_loc):
        x_fm = [ep.tile([128, SP], f32r, name=f"xfm_{dcc}", bufs=2)
                for dcc in range(DC)]
        y1_fm = [ep.tile([128, SP], f32r, name=f"y1fm_{dcc}", bufs=1)
                 for dcc in range(DC)]
        for dc in range(DC):
            nc.vector.tensor_copy(out=x_fm[dc][:, 0:2], in_=zpad[:, :])
            nc.vector.tensor_copy(out=y1_fm[dc][:, 0:2], in_=zpad[:, :])
        m_cols = []
        row0 = row * c.S

        for st in range(NST):
            s0 = st * ST  # within-row token offset
            # ---- gather + mask + transpose the supertile's token tiles ----
            for tt in range(NTT):
                t0 = s0 + tt * 128
                idx = ep.tile([128, 1], i32, name="idx", bufs=4)
                nc.sync.dma_start(out=idx[:], in_=x_in[row0+t0:row0+t0+128, None])
                mc = ep.tile([128, 1], f32, name=f"mc_{st}_{tt}", bufs=2)
                nc.sync.dma_start(out=mc[:], in_=mask[row0+t0:row0+t0+128, None])
                m_cols.append(mc)
                x_tm = ep.tile([128, c.D], f32, name="x_tm", bufs=2)
                nc.gpsimd.indirect_dma_start(
                    out=x_tm[:], out_offset=None, in_=embed[:],
                    in_offset=bass.IndirectOffsetOnAxis(ap=idx[:, :1], axis=0))
                xm = ep.tile([128, c.D], f32, name="xm", bufs=2)
                nc.vector.tensor_scalar_mul(xm[:], x_tm[:], mc[:])
                tp_ps = pe.tile([128, c.D], f32, name="tp_ps", tag="tpb", bufs=2)
                for dc in range(DC):
                    nc.tensor.transpose(out=tp_ps[:, dc*128:(dc+1)*128],
                                        in_=xm[:, dc*128:(dc+1)*128],
                                        identity=ident[:])
                for dc in range(DC):
                    nc.vector.tensor_copy(
                        out=x_fm[dc][:, 2+t0:2+t0+128],
                        in_=tp_ps[:, dc*128:(dc+1)*128])

            # ---- conv1 (fm out) + relu ----
            for dc in range(DC):
                c1_ps = pe.tile([128, ST], f32, name="c1_ps", tag="c1h", bufs=2)
                first = True
                for k in range(K):
                    for dci in range(DC):
                        nc.tensor.matmul(
                            out=c1_ps[:],
                            lhsT=w1r[k*DC+dci][:, dc*128:(dc+1)*128],
                            rhs=x_fm[dci][:, s0+k:s0+k+ST],
                            start=first, stop=(k == K-1 and dci == DC-1))
                        first = False
                nc.scalar.activation(out=y1_fm[dc][:, 2+s0:2+s0+ST],
                                     in_=c1_ps[:], func=AF.Relu,
                                     bias=b1_col[dc][:])

            # ---- conv2 (fm out) + bias + residual -> z ----
            z = []
            for dc in range(DC):
                c2_ps = pe.tile([128, ST], f32, name="c2_ps", tag="c2s", bufs=2)
                first = True
                for k in range(K):
                    for dci in range(DC):
                        nc.tensor.matmul(
                            out=c2_ps[:],
                            lhsT=w2r[k*DC+dci][:, dc*128:(dc+1)*128],
                            rhs=y1_fm[dci][:, s0+k:s0+k+ST],
                            start=first, stop=(k == K-1 and dci == DC-1))
                        first = False
                zt = ep.tile([128, ST], f32r, name=f"z_{dc}", bufs=2)
                nc.vector.scalar_tensor_tensor(
                    out=zt[:], in0=c2_ps[:], scalar=b2_col[dc][:],
                    in1=x_fm[dc][:, 2+s0:2+s0+ST].bitcast(f32),
                    op0=OP.add, op1=OP.add)
                z.append(zt)

            # ---- LN stats via PE matvecs (sum, then sumsq) ----
            st_ps = pe.tile([1, ST], f32, name="st_ps", tag="c2s", bufs=2)
            for dc in range(DC):
                nc.tensor.matmul(out=st_ps[0:1, :], lhsT=ones_col[:],
                                 rhs=z[dc][:], start=(dc == 0),
                                 stop=(dc == DC-1))
            zsq = []
            for dc in range(DC):
                zq = ep.tile([128, ST], f32r, name="zsq", bufs=1)
                nc.scalar.square(zq[:], z[dc][:].bitcast(f32))
                zsq.append(zq)
            sq_ps = pe.tile([1, ST], f32, name="sq_ps", tag="c2s", bufs=2)
            for dc in range(DC):
                nc.tensor.matmul(out=sq_ps[0:1, :], lhsT=ones_col[:],
                                 rhs=zsq[dc][:], start=(dc == 0),
                                 stop=(dc == DC-1))

            # ---- hproj on raw z (LN folded into epilogue) ----
            h_ps = []
            for ec in range(DC):
                hp = pe.tile([128, ST], f32, name=f"h_ps_{ec}", tag="c1h",
                             bufs=2)
                for dc in range(DC):
                    nc.tensor.matmul(out=hp[:],
                                     lhsT=wgr[dc][:, ec*128:(ec+1)*128],
                                     rhs=z[dc][:], start=(dc == 0),
                                     stop=(dc == DC-1))
                h_ps.append(hp)

            # ---- LN scalar chain + broadcasts (overlap hproj) ----
            mu = ep.tile([1, ST], f32r, name="mu", bufs=1)
            nc.scalar.mul(mu[:], st_ps[0:1, :], 1.0 / c.D)
            ms = ep.tile([1, ST], f32, name="ms", bufs=1)
            nc.scalar.mul(ms[:], sq_ps[0:1, :], 1.0 / c.D)
            musq = ep.tile([1, ST], f32, name="musq", bufs=1)
            nc.scalar.square(musq[:], mu[:].bitcast(f32))
            nc.vector.scalar_tensor_tensor(out=ms[:], in0=ms[:],
                                           scalar=LN_EPS, in1=musq[:],
                                           op0=OP.add, op1=OP.subtract)
            nc.vector.reciprocal(ms[:], ms[:])
            rstd_t = ep.tile([1, ST], f32r, name="rstd_t", bufs=1)
            nc.scalar.sqrt(rstd_t[:], ms[:])
            mu_bc = pe.tile([128, ST], f32, name="mu_bc", tag="lg", bufs=2)
            nc.tensor.matmul(out=mu_bc[:], lhsT=ones_row[:], rhs=mu[:],
                             start=True, stop=True)
            rs_bc = pe.tile([128, ST], f32, name="rs_bc", tag="lg", bufs=2)
            nc.tensor.matmul(out=rs_bc[:], lhsT=ones_row[:], rhs=rstd_t[:],
                             start=True, stop=True)

            # ---- epilogue: h = rstd * (Hps - u*mu) (+ c0) ----
            h_fm = []
            for ec in range(DC):
                tmp = ep.tile([128, ST], f32, name="tmp", bufs=2)
                nc.vector.tensor_scalar_mul(tmp[:], mu_bc[:],
                                            u_neg_col[ec][:])
                nc.vector.tensor_add(tmp[:], tmp[:], h_ps[ec][:])
                hf = ep.tile([128, ST], f32r, name=f"hfm_{ec}", bufs=2)
                nc.vector.tensor_mul(hf[:], tmp[:], rs_bc[:])
                if has_c0:
                    nc.vector.tensor_scalar_add(hf[:], hf[:].bitcast(f32),
                                                c0_col[ec][:])
                h_fm.append(hf)

            # ---- logits: [tok, n] = h.T @ HT ----
            for tt in range(NTT):
                mc = m_cols[st * NTT + tt]
                for ns in range(NSL):
                    lg_ps = pe.tile([128, SW], f32, name="lg_ps", tag="lg",
                                    bufs=2)
                    for ec in range(DC):
                        nc.tensor.matmul(
                            out=lg_ps[:],
                            lhsT=h_fm[ec][:, tt*128:(tt+1)*128],
                            rhs=HT[ec][:, ns*SW:(ns+1)*SW],
                            start=(ec == 0), stop=(ec == DC-1))
                    lo = ep.tile([128, SW], f32, name="lo", bufs=2)
                    if ns == 3:
                        nc.vector.tensor_scalar_mul(lo[:], lg_ps[:], mc[:])
                    else:
                        nc.scalar.mul(lo[:], lg_ps[:], mc[:])
                    t0g = row0 + s0 + tt * 128
                    nc.sync.dma_start(out=out[t0g:t0g+128, ns*SW:(ns+1)*SW],
                                      in_=lo[:])
    pe.release()
    ep.release()
    wp.release()
    htp.release()
    const.release()


# ---------------------------------------------------------------------------
# host side
# ---------------------------------------------------------------------------

_CACHE = {}


def _get_program(cfg, has_c0=False):
    key = (cfg.V, cfg.D, cfg.B, cfg.S, cfg.N, cfg.K, cfg.n_cores, has_c0)
    if key not in _CACHE:
        _CACHE[key] = build_program(cfg, has_c0=has_c0)
    return _CACHE[key]


class _Runner:
    """Direct PJRT execution (no donation) so repeated runs are cheap."""

    def __init__(self, nc, n_cores):
        import jax
        from jax.sharding import Mesh, PartitionSpec, NamedSharding
        from jax.experimental.shard_map import shard_map
        from concourse import bass2jax
        bass2jax.install_neuronx_cc_hook()
        self.jax = jax
        self.n_cores = n_cores
        part_name = nc.partition_id_tensor.name if nc.partition_id_tensor else None
        in_names, out_names, out_avals, zero_outs = [], [], [], []
        for alloc in nc.m.functions[0].allocations:
            if not isinstance(alloc, mybir.MemoryLocationSet):
                continue
            name = alloc.memorylocations[0].name
            if alloc.kind == "ExternalInput":
                if name != part_name:
                    in_names.append(name)
            elif alloc.kind == "ExternalOutput":
                out_names.append(name)
                shape = tuple(alloc.tensor_shape)
                dtype = mybir.dt.np(alloc.dtype)
                out_avals.append(jax.core.ShapedArray(shape, dtype))
                zero_outs.append(np.zeros(shape, dtype))
        self.in_names, self.out_names = in_names, out_names
        self.out_avals, self.zero_outs = out_avals, zero_outs
        self.n_params = len(in_names)
        all_in = list(in_names) + list(out_names)
        if part_name:
            all_in.append(part_name)
        out_avals_t, all_in_t, out_names_t = (tuple(out_avals), tuple(all_in),
                                              tuple(out_names))

        def _body(*args):
            operands = list(args)
            if part_name:
                operands.append(bass2jax.partition_id_tensor())
            return tuple(bass2jax._bass_exec_p.bind(
                *operands, out_avals=out_avals_t, in_names=all_in_t,
                out_names=out_names_t, lowering_input_output_aliases=(),
                sim_require_finite=True, sim_require_nnan=True, nc=nc))

        devices = jax.devices()[:n_cores]
        self.mesh = Mesh(np.asarray(devices), ("core",))
        n_io = self.n_params + len(out_names)
        self.sharded = jax.jit(
            shard_map(_body, mesh=self.mesh,
                      in_specs=(PartitionSpec("core"),) * n_io,
                      out_specs=(PartitionSpec("core"),) * len(out_names),
                      check_rep=False),
            keep_unused=True)
        self.shard = NamedSharding(self.mesh, PartitionSpec("core"))

    def set_inputs(self, in_maps):
        jax = self.jax
        per_core = [[np.asarray(m[n]) for n in self.in_names] for m in in_maps]
        concat = [np.concatenate([per_core[cc][i] for cc in range(self.n_cores)],
                                 axis=0) for i in range(self.n_params)]
        concat += [np.zeros((self.n_cores * z.shape[0], *z.shape[1:]), z.dtype)
                   for z in self.zero_outs]
        self.dev_in = [jax.device_put(a, self.shard) for a in concat]
        jax.block_until_ready(self.dev_in)

    def run(self):
        outs = self.sharded(*self.dev_in)
        self.jax.block_until_ready(outs)
        return outs

    def run_np(self):
        outs = self.run()
        return [
            {n: np.asarray(outs[i]).reshape(self.n_cores,
                                            *self.out_avals[i].shape)[cc]
             for i, n in enumerate(self.out_names)}
            for cc in range(self.n_cores)
        ]


_RUNNER = {}


def make_in_maps(cfg, inputs):
    c = cfg
    x_in = np.asarray(inputs['x_in'])
    mask = np.asarray(inputs['mask_in']).astype(np.float32)
    A_T = np.ascontiguousarray(np.asarray(inputs['A_sub']).T)
    hproj_wT = np.ascontiguousarray(np.asarray(inputs['hproj_w']).T)
    gcn_wT = np.ascontiguousarray(np.asarray(inputs['gcn_w']).T)
    shared = {
        'sub_nodes': np.asarray(inputs['sub_nodes']).astype(np.int32),
        'A_subT': A_T.astype(np.float32),
        'embed': np.asarray(inputs['embed']).astype(np.float32),
        'conv1_w': np.asarray(inputs['conv1_w']).astype(np.float32),
        'conv1_b': np.asarray(inputs['conv1_b']).astype(np.float32),
        'conv2_w': np.asarray(inputs['conv2_w']).astype(np.float32),
        'conv2_b': np.asarray(inputs['conv2_b']).astype(np.float32),
        'ln_g': np.asarray(inputs['ln_g']).astype(np.float32),
        'ln_b': np.asarray(inputs['ln_b']).astype(np.float32),
        'gcn_wT': gcn_wT.astype(np.float32),
        'gcn_b': np.asarray(inputs['gcn_b']).astype(np.float32),
        'hproj_wT': hproj_wT.astype(np.float32),
    }
    in_maps = []
    for cc in range(c.n_cores):
        rows = slice(cc * c.B_loc, (cc + 1) * c.B_loc)
        m = dict(shared)
        m['x_in_loc'] = np.ascontiguousarray(
            x_in[rows].reshape(-1)).astype(np.int32)
        m['mask_loc'] = np.ascontiguousarray(mask[rows].reshape(-1))
        in_maps.append(m)
    return in_maps


def kernel(**inputs):
    cfg = Cfg()
    has_c0 = bool(np.any(np.asarray(inputs['ln_b']) != 0))
    nc = _get_program(cfg, has_c0)
    key = id(nc)
    if key not in _RUNNER:
        _RUNNER[key] = _Runner(nc, cfg.n_cores)
    r = _RUNNER[key]
    r.set_inputs(make_in_maps(cfg, inputs))
    res = r.run_np()
    out = np.concatenate(
        [res[cc]['logits_loc'].reshape(cfg.B_loc, cfg.S, cfg.N)
         for cc in range(cfg.n_cores)], axis=0)
    return out



# revision 14
# speedup vs baseline: 1.4351x; 1.4351x over previous
"""Trainium2 Bass kernel for nn_CandidateSubgraphTFModel (gnn_message_passing).

Strategy (8 NeuronCores, SPMD, no collectives):
  - Data-parallel over batch B=32 -> 4 rows/core for the TCN/encode path.
  - GCN (candidate_embed) replicated on every core; hproj fused into the
    candidate matrix G = (g*hproj)^T @ H^T so the per-token hproj matmul
    disappears: logits = rstd * ((z - mu) @ G) * mask.
  - Degree vector d = (rowsum(A)+1)^-1/2 precomputed on host; d_j folded
    into X, d_i folded into the m1 PSUM->SBUF copy.
  - All matmuls fp32r (1 cyc/row on PE for moving>=256).
  - LN stats via ones-matvec on PE; rstd computed in column form (cheap
    [128,4] vector math) and applied in the output scale together with
    the mask - nothing serial on the supertile critical path.

kernel(**inputs) takes FULL inputs (as in reference.setup_inputs()) and
returns the FULL [32, 1024, 2048] logits.
"""
import sys
sys.path.insert(0, '/opt/trn_rl_repo')
import numpy as np

import concourse.bass as bass
import concourse.bacc as bacc
import concourse.tile as tile
from concourse import mybir
from concourse.masks import make_identity

f32 = mybir.dt.float32
f32r = mybir.dt.float32r
i32 = mybir.dt.int32
AF = mybir.ActivationFunctionType
OP = mybir.AluOpType

LN_EPS = 1e-5


class Cfg:
    def __init__(self, V=50000, D=512, B=32, S=1024, N=2048, K=3, n_cores=8):
        self.V, self.D, self.B, self.S, self.N, self.K = V, D, B, S, N, K
        self.n_cores = n_cores
        self.B_loc = B // n_cores
        self.DC = D // 128          # feature chunks
        self.ST = min(512, S)       # supertile (tokens)
        self.NTT = self.ST // 128   # token-tiles per supertile
        self.NST = S // self.ST     # supertiles per row
        self.NJC = N // 128         # GCN j chunks
        self.SW = min(512, N)       # slice width over N
        self.NSL = N // self.SW     # slices over N
        self.TOK = self.B_loc * S   # tokens per core


def build_program(cfg, reps=1, has_c0=False):
    c = cfg
    nc = bacc.Bacc("TRN2", target_bir_lowering=False, debug=False)

    x_in = nc.dram_tensor("x_in_loc", [c.TOK], i32, kind="ExternalInput").ap()
    mask = nc.dram_tensor("mask_loc", [c.TOK], f32, kind="ExternalInput").ap()
    sub_nodes = nc.dram_tensor("sub_nodes", [c.N], i32, kind="ExternalInput").ap()
    A_T = nc.dram_tensor("A_subT", [c.N, c.N], f32, kind="ExternalInput").ap()
    embed = nc.dram_tensor("embed", [c.V, c.D], f32, kind="ExternalInput").ap()
    c1w = nc.dram_tensor("conv1_w", [c.K, c.D, c.D], f32, kind="ExternalInput").ap()
    c1b = nc.dram_tensor("conv1_b", [c.D], f32, kind="ExternalInput").ap()
    c2w = nc.dram_tensor("conv2_w", [c.K, c.D, c.D], f32, kind="ExternalInput").ap()
    c2b = nc.dram_tensor("conv2_b", [c.D], f32, kind="ExternalInput").ap()
    gwT = nc.dram_tensor("gcn_wT", [c.D, c.D], f32, kind="ExternalInput").ap()
    gb = nc.dram_tensor("gcn_b", [c.D], f32, kind="ExternalInput").ap()
    hwg = nc.dram_tensor("hproj_wg", [c.D, c.D], f32, kind="ExternalInput").ap()
    hb = nc.dram_tensor("hproj_b", [c.D], f32, kind="ExternalInput").ap()
    d_cm = nc.dram_tensor("d_col_mat", [128, c.NJC], f32, kind="ExternalInput").ap()
    d_rv = nc.dram_tensor("d_row", [1, c.N], f32, kind="ExternalInput").ap()
    out = nc.dram_tensor("logits_loc", [c.TOK, c.N], f32, kind="ExternalOutput").ap()

    with tile.TileContext(nc) as tc:
        for _ in range(reps):
            build_body(tc, c, x_in, mask, sub_nodes, A_T, embed, c1w, c1b,
                       c2w, c2b, gwT, gb, hwg, hb, d_cm, d_rv, out, has_c0)
    nc.compile()
    return nc


def build_body(tc, c, x_in, mask, sub_nodes, A_T, embed, c1w, c1b, c2w, c2b,
               gwT, gb, hwg, hb, d_cm, d_rv, out, has_c0=False):
    nc = tc.nc
    DC, K, ST, NTT, NST, NJC, SW, NSL = (c.DC, c.K, c.ST, c.NTT, c.NST,
                                         c.NJC, c.SW, c.NSL)

    # ---------------- pools ----------------
    const = tc.alloc_tile_pool(name="const", bufs=1)
    wp = tc.alloc_tile_pool(name="wp", bufs=1)

    # ---------------- constants ----------------
    ident = const.tile([128, 128], f32)
    make_identity(nc, ident[:])
    ones_f = const.tile([128, 1], f32)
    nc.vector.memset(ones_f[:], 1.0)
    onesr_f = const.tile([1, 128], f32)
    nc.vector.memset(onesr_f[:], 1.0)
    ones_col = const.tile([128, 1], f32r)
    nc.vector.tensor_copy(out=ones_col[:], in_=ones_f[:])
    ones_row = const.tile([1, 128], f32r)
    nc.vector.tensor_copy(out=ones_row[:], in_=onesr_f[:])
    zpad = const.tile([128, 2], f32)
    nc.vector.memset(zpad[:], 0.0)
    identr = const.tile([128, 128], f32r)
    nc.vector.tensor_copy(out=identr[:], in_=ident[:])

    d_col = const.tile([128, NJC], f32)
    nc.sync.dma_start(out=d_col[:], in_=d_cm[:, :])
    d_rowr = const.tile([1, c.N], f32r)

    def load_cols(dram_vec, name):
        cols = []
        for dc in range(DC):
            t = const.tile([128, 1], f32, name=f"{name}_{dc}")
            nc.sync.dma_start(out=t[:], in_=dram_vec[dc*128:(dc+1)*128, None])
            cols.append(t)
        return cols

    b1_col = load_cols(c1b, "b1")
    b2_col = load_cols(c2b, "b2")
    gb_col = load_cols(gb, "gb")
    hb_col = load_cols(hb, "hb") if has_c0 else None

    # conv weights -> f32r tiles [128(din), D(dout)] per (k, din_chunk)
    stage = tc.alloc_tile_pool(name="stage", bufs=3)
    d_row_f = stage.tile([1, c.N], f32, name="d_row_f", bufs=1)
    nc.sync.dma_start(out=d_row_f[:], in_=d_rv[:, :])
    nc.vector.tensor_copy(out=d_rowr[:], in_=d_row_f[:])
    w1r, w2r = [], []
    for (wsrc, wdst) in ((c1w, w1r), (c2w, w2r)):
        for k in range(K):
            for dc in range(DC):
                st_ = stage.tile([128, c.D], f32, name="wstg")
                nc.sync.dma_start(out=st_[:], in_=wsrc[k, dc*128:(dc+1)*128, :])
                wr = wp.tile([128, c.D], f32r, name=f"w_{len(wdst)}_{id(wdst)%97}")
                nc.vector.tensor_copy(out=wr[:], in_=st_[:])
                wdst.append(wr)

    # gcn weights f32r [128(d), D(e)]
    gwr = []
    for dc in range(DC):
        st_ = stage.tile([128, c.D], f32, name="wstg3")
        nc.sync.dma_start(out=st_[:], in_=gwT[dc*128:(dc+1)*128, :])
        gr = wp.tile([128, c.D], f32r, name=f"gw_{dc}")
        nc.vector.tensor_copy(out=gr[:], in_=st_[:])
        gwr.append(gr)

    # hproj*g (host-folded) f32r [128(e), D(d)]
    hwgr = []
    for ec in range(DC):
        st_ = stage.tile([128, c.D], f32, name="wstg4")
        nc.sync.dma_start(out=st_[:], in_=hwg[ec*128:(ec+1)*128, :])
        hr = wp.tile([128, c.D], f32r, name=f"hwg_{ec}")
        nc.vector.tensor_copy(out=hr[:], in_=st_[:])
        hwgr.append(hr)
    stage.release()

    # ---------------- encode SBUF (persists all rows) ----------------
    epx = tc.alloc_tile_pool(name="epx", bufs=1)
    SP = c.S + 2  # row buffer width (2 zero pad cols at left)

    x_fm_rows = {}   # row -> list of DC tiles
    m_cols_rows = {}

    def gather_piece(row, tt, psum_pool, x_fm, m_cols):
        """Gather+transpose token tile tt (of 8) of `row` into x_fm."""
        row0 = row * c.S
        t0 = tt * 128
        if tt == 0:
            for dc in range(DC):
                nc.vector.tensor_copy(out=x_fm[dc][:, 0:2], in_=zpad[:, :])
        idx = epx.tile([128, 1], i32, name="idx", tag="idx", bufs=4)
        nc.sync.dma_start(out=idx[:], in_=x_in[row0+t0:row0+t0+128, None])
        mc = epx.tile([128, 1], f32, name="mc", tag="mc", bufs=16)
        nc.sync.dma_start(out=mc[:], in_=mask[row0+t0:row0+t0+128, None])
        m_cols.append(mc)
        x_tm = epx.tile([128, c.D], f32, name="x_tm", tag="x_tm", bufs=2)
        nc.gpsimd.indirect_dma_start(
            out=x_tm[:], out_offset=None, in_=embed[:],
            in_offset=bass.IndirectOffsetOnAxis(ap=idx[:, :1], axis=0))
        xm = epx.tile([128, c.D], f32r, name="xm", tag="xm", bufs=2)
        nc.vector.tensor_scalar_mul(xm[:], x_tm[:], mc[:])
        tp_ps = psum_pool.tile([128, c.D], f32r, name="tp_ps", tag="tp", bufs=1)
        for dc in range(DC):
            nc.tensor.transpose(out=tp_ps[:, dc*128:(dc+1)*128],
                                in_=xm[:, dc*128:(dc+1)*128],
                                identity=identr[:])
        for dc in range(DC):
            nc.vector.tensor_copy(out=x_fm[dc][:, 2+t0:2+t0+128],
                                  in_=tp_ps[:, dc*128:(dc+1)*128])

    def new_row_bufs(row):
        x_fm = [epx.tile([128, SP], f32r, name=f"xfm_{dcc}", tag=f"xfm{dcc}",
                         bufs=1) for dcc in range(DC)]
        x_fm_rows[row] = x_fm
        m_cols_rows[row] = []
        return x_fm, m_cols_rows[row]

    # ---------------- row 0 pre-gather (overlaps GCN DMA warmup) --------
    pg0 = tc.alloc_tile_pool(name="pg0", bufs=1, space="PSUM")
    x_fm0, m_cols0 = new_row_bufs(0)
    for tt in range(NTT * NST):
        gather_piece(0, tt, pg0, x_fm0, m_cols0)
    pg0.release()

    # ================= GCN phase (replicated) =================
    gtp = tc.alloc_tile_pool(name="gtp", bufs=1)
    G = [gtp.tile([128, c.N], f32r, name=f"G_{dc}") for dc in range(DC)]
    r0_row = gtp.tile([1, c.N], f32r, name="r0_row") if has_c0 else None

    gp = tc.alloc_tile_pool(name="gp", bufs=1)
    pgm = tc.alloc_tile_pool(name="pgm", bufs=1, space="PSUM")

    # gather X = embed[sub_nodes]; fold d_j in the rounding copy
    Xr = []
    for jc in range(NJC):
        sidx = gp.tile([128, 1], i32, name="sidx", tag="sidx", bufs=4)
        nc.sync.dma_start(out=sidx[:], in_=sub_nodes[jc*128:(jc+1)*128, None])
        xstg = gp.tile([128, c.D], f32, name="xstg", tag="xstg", bufs=2)
        nc.gpsimd.indirect_dma_start(
            out=xstg[:], out_offset=None, in_=embed[:],
            in_offset=bass.IndirectOffsetOnAxis(ap=sidx[:, :1], axis=0))
        xt = gp.tile([128, c.D], f32r, name=f"Xg_{jc}", tag=f"Xr{jc}")
        nc.vector.tensor_scalar_mul(xt[:], xstg[:], d_col[:, jc:jc+1])
        Xr.append(xt)

    # per i-slice: m1 = X~^T @ A_hatT[:, islice]; fold d_i in psum->sbuf copy
    for sl in range(NSL):
        m1_ps = [pgm.tile([128, SW], f32, name=f"m1p_{dc}", tag=f"gm{dc}",
                          bufs=1) for dc in range(DC)]
        for jc in range(NJC):
            a2 = gp.tile([128, SW], f32, name="a2", tag="a2", bufs=2)
            nc.sync.dma_start(out=a2[:], in_=A_T[jc*128:(jc+1)*128,
                                               sl*SW:(sl+1)*SW])
            if jc * 128 >= sl * SW and (jc + 1) * 128 <= (sl + 1) * SW:
                off = jc * 128 - sl * SW
                nc.vector.tensor_tensor(
                    out=a2[:, off:off+128], in0=a2[:, off:off+128],
                    in1=ident[:], op=OP.add)
            ar2 = gp.tile([128, SW], f32r, name="ar2", tag="ar2", bufs=2)
            nc.vector.tensor_copy(out=ar2[:], in_=a2[:])
            for dc in range(DC):
                nc.tensor.matmul(out=m1_ps[dc][:],
                                 lhsT=Xr[jc][:, dc*128:(dc+1)*128],
                                 rhs=ar2[:], start=(jc == 0),
                                 stop=(jc == NJC - 1))
        # d_i broadcast for this slice
        db_ps = pgm.tile([128, SW], f32, name="db_ps", tag="gsc", bufs=2)
        nc.tensor.matmul(out=db_ps[:], lhsT=ones_row[:],
                         rhs=d_rowr[0:1, sl*SW:(sl+1)*SW], start=True,
                         stop=True)
        db_sb = gp.tile([128, SW], f32, name="db_sb", tag="db", bufs=2)
        nc.vector.tensor_copy(out=db_sb[:], in_=db_ps[:])
        m1s = []
        for dc in range(DC):
            m1t = gp.tile([128, SW], f32r, name=f"m1s_{dc}", tag=f"m1s{dc}",
                          bufs=1)
            nc.vector.tensor_mul(m1t[:], m1_ps[dc][:], db_sb[:])
            m1s.append(m1t)
        HTs = []
        for ec in range(DC):
            h2_ps = pgm.tile([128, SW], f32, name="h2_ps", tag="gsc", bufs=2)
            for dc in range(DC):
                nc.tensor.matmul(out=h2_ps[:],
                                 lhsT=gwr[dc][:, ec*128:(ec+1)*128],
                                 rhs=m1s[dc][:], start=(dc == 0),
                                 stop=(dc == DC - 1))
            ht = gp.tile([128, SW], f32r, name=f"HT_{ec}", tag=f"ht{ec}",
                         bufs=2)
            nc.scalar.activation(out=ht[:], in_=h2_ps[:],
                                 func=AF.Relu, bias=gb_col[ec][:])
            HTs.append(ht)
        # G slice for this sl: G[dc][:, sl] = sum_ec hwgr[ec][:,dc] @ HTs[ec]
        for dc in range(DC):
            g_ps = pgm.tile([128, SW], f32, name="g_ps", tag="gg", bufs=2)
            for ec in range(DC):
                nc.tensor.matmul(out=g_ps[:],
                                 lhsT=hwgr[ec][:, dc*128:(dc+1)*128],
                                 rhs=HTs[ec][:], start=(ec == 0),
                                 stop=(ec == DC - 1))
            nc.vector.tensor_copy(out=G[dc][:, sl*SW:(sl+1)*SW], in_=g_ps[:])
        if has_c0:
            r0_ps = pgm.tile([1, SW], f32, name="r0_ps", tag="gsc", bufs=2)
            for ec in range(DC):
                nc.tensor.matmul(out=r0_ps[0:1, :], lhsT=hb_col[ec][:],
                                 rhs=HTs[ec][:], start=(ec == 0),
                                 stop=(ec == DC - 1))
            nc.vector.tensor_copy(out=r0_row[0:1, sl*SW:(sl+1)*SW],
                                  in_=r0_ps[:])
    gp.release()
    pgm.release()

    # ================= encode + logits =================
    ep = tc.alloc_tile_pool(name="ep", bufs=1)
    pe = tc.alloc_tile_pool(name="pe", bufs=1, space="PSUM")

    for row in range(c.B_loc):
        x_fm = x_fm_rows[row]
        m_cols = m_cols_rows[row]
        row0 = row * c.S

        y1_fm = [ep.tile([128, SP], f32r, name=f"y1fm_{dcc}", tag=f"y1{dcc}",
                         bufs=1) for dcc in range(DC)]
        for dc in range(DC):
            nc.vector.tensor_copy(out=y1_fm[dc][:, 0:2], in_=zpad[:, :])

        # ---- conv1 both supertiles ----
        for dcout in range(DC):
            for st in range(NST):
                s0 = st * ST
                c1_ps = pe.tile([128, ST], f32, name="c1_ps", tag="c1", bufs=2)
                first = True
                for k in range(K):
                    for dci in range(DC):
                        nc.tensor.matmul(
                            out=c1_ps[:],
                            lhsT=w1r[k*DC+dci][:, dcout*128:(dcout+1)*128],
                            rhs=x_fm[dci][:, s0+k:s0+k+ST],
                            start=first, stop=(k == K-1 and dci == DC-1))
                        first = False
                nc.scalar.activation(out=y1_fm[dcout][:, 2+s0:2+s0+ST],
                                     in_=c1_ps[:], func=AF.Relu,
                                     bias=b1_col[dcout][:])

        # ---- conv2 + stats + LN cols, per supertile ----
        z_st, sc_st = [], []
        for st in range(NST):
            s0 = st * ST
            z, zq = [], []
            for dcout in range(DC):
                c2_ps = pe.tile([128, ST], f32, name="c2_ps", tag="c2", bufs=2)
                first = True
                for k in range(K):
                    for dci in range(DC):
                        nc.tensor.matmul(
                            out=c2_ps[:],
                            lhsT=w2r[k*DC+dci][:, dcout*128:(dcout+1)*128],
                            rhs=y1_fm[dci][:, s0+k:s0+k+ST],
                            start=first, stop=(k == K-1 and dci == DC-1))
                        first = False
                zt = ep.tile([128, ST], f32r, name=f"z_{dcout}",
                             tag=f"z{dcout}", bufs=1)
                nc.vector.scalar_tensor_tensor(
                    out=zt[:], in0=c2_ps[:], scalar=b2_col[dcout][:],
                    in1=x_fm[dcout][:, 2+s0:2+s0+ST].bitcast(f32),
                    op0=OP.add, op1=OP.add)
                z.append(zt)
                zqt = ep.tile([128, ST], f32r, name="zsq", tag="zsq", bufs=4)
                nc.scalar.square(zqt[:], zt[:].bitcast(f32))
                zq.append(zqt)
            st_ps = pe.tile([1, ST], f32, name="st_ps", tag="sa", bufs=1)
            for dc in range(DC):
                nc.tensor.matmul(out=st_ps[0:1, :], lhsT=ones_col[:],
                                 rhs=z[dc][:], start=(dc == 0),
                                 stop=(dc == DC-1))
            sq_ps = pe.tile([1, ST], f32, name="sq_ps", tag="c2", bufs=2)
            for dc in range(DC):
                nc.tensor.matmul(out=sq_ps[0:1, :], lhsT=ones_col[:],
                                 rhs=zq[dc][:], start=(dc == 0),
                                 stop=(dc == DC-1))
            mu_row = ep.tile([1, ST], f32, name="mu_row", tag="mu_row",
                             bufs=2)
            nc.scalar.mul(mu_row[:], st_ps[0:1, :], 1.0 / c.D)
            mu_rowr = ep.tile([1, ST], f32r, name="mu_rowr", tag="mu_rowr",
                              bufs=2)
            nc.vector.tensor_copy(out=mu_rowr[:], in_=mu_row[:])
            ms_row = ep.tile([1, ST], f32, name="ms_row", tag="ms_row",
                             bufs=2)
            nc.scalar.mul(ms_row[:], sq_ps[0:1, :], 1.0 / c.D)
            # transpose stats rows -> columns [128, NTT each]
            tr_ps = pe.tile([128, 2*NTT], f32, name="tr_ps", tag="c2",
                            bufs=2)
            for tt in range(NTT):
                nc.tensor.transpose(out=tr_ps[:, tt:tt+1],
                                    in_=mu_row[0:1, tt*128:(tt+1)*128],
                                    identity=ident[0:1, 0:1])
            for tt in range(NTT):
                nc.tensor.transpose(out=tr_ps[:, NTT+tt:NTT+tt+1],
                                    in_=ms_row[0:1, tt*128:(tt+1)*128],
                                    identity=ident[0:1, 0:1])
            # mu broadcast ([128, ST], K=1 matmul)
            mu_bc = pe.tile([128, ST], f32, name="mu_bc", tag="c2", bufs=2)
            nc.tensor.matmul(out=mu_bc[:], lhsT=ones_row[:], rhs=mu_rowr[:],
                             start=True, stop=True)
            # zc = z - mu
            zc = []
            for dc in range(DC):
                zct = ep.tile([128, ST], f32r, name=f"zc_{dc}", tag=f"zc{dc}",
                              bufs=2)
                nc.vector.scalar_tensor_tensor(
                    out=zct[:], in0=mu_bc[:], scalar=-1.0,
                    in1=z[dc][:].bitcast(f32), op0=OP.mult, op1=OP.add)
                zc.append(zct)
            # var/rstd in columns
            musq = ep.tile([128, NTT], f32, name="musq", tag="musq", bufs=2)
            nc.scalar.square(musq[:], tr_ps[:, 0:NTT])
            var_c = ep.tile([128, NTT], f32, name="var_c", tag="var_c", bufs=2)
            nc.vector.tensor_tensor(out=var_c[:], in0=tr_ps[:, NTT:2*NTT],
                                    in1=musq[:], op=OP.subtract)
            nc.vector.tensor_scalar_add(var_c[:], var_c[:], LN_EPS)
            nc.vector.reciprocal(var_c[:], var_c[:])
            rstd_c = ep.tile([128, NTT], f32, name="rstd_c", tag="rstd_c",
                             bufs=2)
            nc.scalar.sqrt(rstd_c[:], var_c[:])
            sc_cols = []
            for tt in range(NTT):
                sc = ep.tile([128, 1], f32, name="sc", tag="sc", bufs=16)
                nc.vector.tensor_scalar_mul(sc[:], rstd_c[:, tt:tt+1],
                                            m_cols[st*NTT+tt][:])
                sc_cols.append(sc)
            std_rows = None
            if has_c0:
                std_c = ep.tile([128, NTT], f32, name="std_c", tag="std_c",
                                bufs=2)
                nc.vector.tensor_tensor(out=std_c[:], in0=rstd_c[:],
                                        in1=var_c[:], op=OP.divide)
                std_rows = []
                for tt in range(NTT):
                    sr_ps = pe.tile([1, 128], f32, name="sr_ps", tag="sa",
                                    bufs=1)
                    nc.tensor.transpose(out=sr_ps[:],
                                        in_=std_c[:, tt:tt+1],
                                        identity=ident[0:1, 0:1])
                    sr = ep.tile([1, 128], f32r, name="sr", tag="sr", bufs=8)
                    nc.vector.tensor_copy(out=sr[:], in_=sr_ps[:])
                    std_rows.append(sr)
            z_st.append(zc)
            sc_st.append((sc_cols, std_rows))

        # ---- logits (+ next-row gather interleaved) ----
        nxt = None
        if row + 1 < c.B_loc:
            nxt = new_row_bufs(row + 1)
        piece = 0
        for st in range(NST):
            s0 = st * ST
            zc = z_st[st]
            sc_cols, std_rows = sc_st[st]
            for tt in range(NTT):
                for ns in range(NSL):
                    lg_ps = pe.tile([128, SW], f32, name="lg_ps", tag="lg",
                                    bufs=2)
                    for ec in range(DC):
                        nc.tensor.matmul(
                            out=lg_ps[:],
                            lhsT=zc[ec][:, tt*128:(tt+1)*128],
                            rhs=G[ec][:, ns*SW:(ns+1)*SW],
                            start=(ec == 0),
                            stop=(ec == DC-1 and not has_c0))
                        if has_c0 and ec == DC - 1:
                            nc.tensor.matmul(
                                out=lg_ps[:], lhsT=std_rows[tt][:],
                                rhs=r0_row[0:1, ns*SW:(ns+1)*SW],
                                start=False, stop=True)
                    lo = ep.tile([128, SW], f32, name="lo", tag="lo", bufs=2)
                    nc.scalar.mul(lo[:], lg_ps[:], sc_cols[tt][:])
                    t0g = row0 + s0 + tt * 128
                    nc.scalar.dma_start(out=out[t0g:t0g+128, ns*SW:(ns+1)*SW],
                                        in_=lo[:])
                if nxt is not None:
                    gather_piece(row + 1, piece, pe, nxt[0], nxt[1])
                piece += 1
    pe.release()
    ep.release()
    gtp.release()
    epx.release()
    wp.release()
    const.release()


# ---------------------------------------------------------------------------
# host side
# ---------------------------------------------------------------------------

_CACHE = {}


def _get_program(cfg, has_c0=False):
    key = (cfg.V, cfg.D, cfg.B, cfg.S, cfg.N, cfg.K, cfg.n_cores, has_c0)
    if key not in _CACHE:
        _CACHE[key] = build_program(cfg, has_c0=has_c0)
    return _CACHE[key]


class _Runner:
    """Direct PJRT execution (no donation) so repeated runs are cheap."""

    def __init__(self, nc, n_cores):
        import jax
        from jax.sharding import Mesh, PartitionSpec, NamedSharding
        from jax.experimental.shard_map import shard_map
        from concourse import bass2jax
        bass2jax.install_neuronx_cc_hook()
        self.jax = jax
        self.n_cores = n_cores
        part_name = nc.partition_id_tensor.name if nc.partition_id_tensor else None
        in_names, out_names, out_avals, zero_outs = [], [], [], []
        for alloc in nc.m.functions[0].allocations:
            if not isinstance(alloc, mybir.MemoryLocationSet):
                continue
            name = alloc.memorylocations[0].name
            if alloc.kind == "ExternalInput":
                if name != part_name:
                    in_names.append(name)
            elif alloc.kind == "ExternalOutput":
                out_names.append(name)
                shape = tuple(alloc.tensor_shape)
                dtype = mybir.dt.np(alloc.dtype)
                out_avals.append(jax.core.ShapedArray(shape, dtype))
                zero_outs.append(np.zeros(shape, dtype))
        self.in_names, self.out_names = in_names, out_names
        self.out_avals, self.zero_outs = out_avals, zero_outs
        self.n_params = len(in_names)
        all_in = list(in_names) + list(out_names)
        if part_name:
            all_in.append(part_name)
        out_avals_t, all_in_t, out_names_t = (tuple(out_avals), tuple(all_in),
                                              tuple(out_names))

        def _body(*args):
            operands = list(args)
            if part_name:
                operands.append(bass2jax.partition_id_tensor())
            return tuple(bass2jax._bass_exec_p.bind(
                *operands, out_avals=out_avals_t, in_names=all_in_t,
                out_names=out_names_t, lowering_input_output_aliases=(),
                sim_require_finite=True, sim_require_nnan=True, nc=nc))

        devices = jax.devices()[:n_cores]
        self.mesh = Mesh(np.asarray(devices), ("core",))
        n_io = self.n_params + len(out_names)
        self.sharded = jax.jit(
            shard_map(_body, mesh=self.mesh,
                      in_specs=(PartitionSpec("core"),) * n_io,
                      out_specs=(PartitionSpec("core"),) * len(out_names),
                      check_rep=False),
            keep_unused=True)
        self.shard = NamedSharding(self.mesh, PartitionSpec("core"))

    def set_inputs(self, in_maps):
        jax = self.jax
        per_core = [[np.asarray(m[n]) for n in self.in_names] for m in in_maps]
        concat = [np.concatenate([per_core[cc][i] for cc in range(self.n_cores)],
                                 axis=0) for i in range(self.n_params)]
        concat += [np.zeros((self.n_cores * z.shape[0], *z.shape[1:]), z.dtype)
                   for z in self.zero_outs]
        self.dev_in = [jax.device_put(a, self.shard) for a in concat]
        jax.block_until_ready(self.dev_in)

    def run(self):
        outs = self.sharded(*self.dev_in)
        self.jax.block_until_ready(outs)
        return outs

    def run_np(self):
        outs = self.run()
        return [
            {n: np.asarray(outs[i]).reshape(self.n_cores,
                                            *self.out_avals[i].shape)[cc]
             for i, n in enumerate(self.out_names)}
            for cc in range(self.n_cores)
        ]


_RUNNER = {}


def make_in_maps(cfg, inputs):
    c = cfg
    x_in = np.asarray(inputs['x_in'])
    mask = np.asarray(inputs['mask_in']).astype(np.float32)
    A = np.asarray(inputs['A_sub']).astype(np.float32)
    A_T = np.ascontiguousarray(A.T)
    ln_g = np.asarray(inputs['ln_g']).astype(np.float32)
    ln_b = np.asarray(inputs['ln_b']).astype(np.float32)
    hproj_w = np.asarray(inputs['hproj_w']).astype(np.float32)
    gcn_wT = np.ascontiguousarray(np.asarray(inputs['gcn_w']).T)
    # degree vector d = clip(rowsum(A)+1, 1e-6)^-0.5 (normalization prep)
    d = np.clip(A.sum(axis=1) + 1.0, 1e-6, None) ** -0.5
    d = d.astype(np.float32)
    d_col_mat = np.ascontiguousarray(d.reshape(c.NJC, 128).T)
    d_row = np.ascontiguousarray(d[None, :])
    hproj_wg = np.ascontiguousarray(hproj_w * ln_g[None, :])
    hproj_b = np.ascontiguousarray(hproj_w @ ln_b)
    shared = {
        'sub_nodes': np.asarray(inputs['sub_nodes']).astype(np.int32),
        'A_subT': A_T,
        'embed': np.asarray(inputs['embed']).astype(np.float32),
        'conv1_w': np.asarray(inputs['conv1_w']).astype(np.float32),
        'conv1_b': np.asarray(inputs['conv1_b']).astype(np.float32),
        'conv2_w': np.asarray(inputs['conv2_w']).astype(np.float32),
        'conv2_b': np.asarray(inputs['conv2_b']).astype(np.float32),
        'gcn_wT': gcn_wT.astype(np.float32),
        'gcn_b': np.asarray(inputs['gcn_b']).astype(np.float32),
        'hproj_wg': hproj_wg,
        'hproj_b': hproj_b.astype(np.float32),
        'd_col_mat': d_col_mat,
        'd_row': d_row,
    }
    in_maps = []
    for cc in range(c.n_cores):
        rows = slice(cc * c.B_loc, (cc + 1) * c.B_loc)
        m = dict(shared)
        m['x_in_loc'] = np.ascontiguousarray(
            x_in[rows].reshape(-1)).astype(np.int32)
        m['mask_loc'] = np.ascontiguousarray(mask[rows].reshape(-1))
        in_maps.append(m)
    return in_maps


def kernel(**inputs):
    cfg = Cfg()
    has_c0 = bool(np.any(np.asarray(inputs['ln_b']) != 0))
    nc = _get_program(cfg, has_c0)
    key = id(nc)
    if key not in _RUNNER:
        _RUNNER[key] = _Runner(nc, cfg.n_cores)
    r = _RUNNER[key]
    r.set_inputs(make_in_maps(cfg, inputs))
    res = r.run_np()
    out = np.concatenate(
        [res[cc]['logits_loc'].reshape(cfg.B_loc, cfg.S, cfg.N)
         for cc in range(cfg.n_cores)], axis=0)
    return out


# revision 18
# speedup vs baseline: 1.9790x; 1.3790x over previous
"""Trainium2 Bass kernel for nn_CandidateSubgraphTFModel (gnn_message_passing).

Strategy (8 NeuronCores, SPMD, no collectives):
  - Data-parallel over batch B=32 -> 4 rows/core for the TCN/encode path.
  - GCN (candidate_embed) replicated on every core; hproj fused into the
    candidate matrix G = (g*hproj)^T @ H^T so the per-token hproj matmul
    disappears: logits = rstd * ((z - mu) @ G) * mask.
  - Degree vector d = (rowsum(A)+1)^-1/2 precomputed on host; d_j folded
    into X, d_i folded into the m1 PSUM->SBUF copy.
  - All matmuls fp32r (1 cyc/row on PE for moving>=256).
  - LN stats via ones-matvec on PE; rstd computed in column form (cheap
    [128,4] vector math) and applied in the output scale together with
    the mask - nothing serial on the supertile critical path.

kernel(**inputs) takes FULL inputs (as in reference.setup_inputs()) and
returns the FULL [32, 1024, 2048] logits.
"""
import sys
sys.path.insert(0, '/opt/trn_rl_repo')
import numpy as np

import concourse.bass as bass
import concourse.bacc as bacc
import concourse.tile as tile
from concourse import mybir
from concourse.masks import make_identity

f32 = mybir.dt.float32
f32r = mybir.dt.float32r
i32 = mybir.dt.int32
AF = mybir.ActivationFunctionType
OP = mybir.AluOpType

LN_EPS = 1e-5


class Cfg:
    def __init__(self, V=50000, D=512, B=32, S=1024, N=2048, K=3, n_cores=8):
        self.V, self.D, self.B, self.S, self.N, self.K = V, D, B, S, N, K
        self.n_cores = n_cores
        self.B_loc = B // n_cores
        self.DC = D // 128          # feature chunks
        self.ST = min(512, S)       # supertile (tokens)
        self.NTT = self.ST // 128   # token-tiles per supertile
        self.NST = S // self.ST     # supertiles per row
        self.NJC = N // 128         # GCN j chunks
        self.SW = min(512, N)       # slice width over N
        self.NSL = N // self.SW     # slices over N
        self.TOK = self.B_loc * S   # tokens per core


def build_program(cfg, reps=1, has_c0=False):
    c = cfg
    nc = bacc.Bacc("TRN2", target_bir_lowering=False, debug=False,
                   num_devices=cfg.n_cores)

    x_in = nc.dram_tensor("x_in_loc", [c.TOK], i32, kind="ExternalInput").ap()
    mask = nc.dram_tensor("mask_loc", [c.TOK], f32, kind="ExternalInput").ap()
    sub_nodes = nc.dram_tensor("sub_nodes", [c.N], i32, kind="ExternalInput").ap()
    A_T = nc.dram_tensor("A_subT", [c.N, c.N // c.n_cores], f32,
                         kind="ExternalInput").ap()
    embed = nc.dram_tensor("embed", [c.V, c.D], f32, kind="ExternalInput").ap()
    c1w = nc.dram_tensor("conv1_w", [c.K, c.D, c.D], f32, kind="ExternalInput").ap()
    c1b = nc.dram_tensor("conv1_b", [c.D], f32, kind="ExternalInput").ap()
    c2w = nc.dram_tensor("conv2_w", [c.K, c.D, c.D], f32, kind="ExternalInput").ap()
    c2b = nc.dram_tensor("conv2_b", [c.D], f32, kind="ExternalInput").ap()
    gwT = nc.dram_tensor("gcn_wT", [c.D, c.D], f32, kind="ExternalInput").ap()
    gb = nc.dram_tensor("gcn_b", [c.D], f32, kind="ExternalInput").ap()
    hwg = nc.dram_tensor("hproj_wg", [c.D, c.D], f32, kind="ExternalInput").ap()
    hb = nc.dram_tensor("hproj_b", [c.D], f32, kind="ExternalInput").ap()
    d_cm = nc.dram_tensor("d_col_mat", [128, c.NJC], f32, kind="ExternalInput").ap()
    d_rv = nc.dram_tensor("d_row", [1, c.N // c.n_cores], f32,
                          kind="ExternalInput").ap()
    out = nc.dram_tensor("logits_loc", [c.TOK, c.N], f32, kind="ExternalOutput").ap()

    with tile.TileContext(nc) as tc:
        for _ in range(reps):
            build_body(tc, c, x_in, mask, sub_nodes, A_T, embed, c1w, c1b,
                       c2w, c2b, gwT, gb, hwg, hb, d_cm, d_rv, out, has_c0)
    nc.compile()
    return nc


def build_body(tc, c, x_in, mask, sub_nodes, A_T, embed, c1w, c1b, c2w, c2b,
               gwT, gb, hwg, hb, d_cm, d_rv, out, has_c0=False):
    nc = tc.nc
    DC, K, ST, NTT, NST, NJC, SW, NSL = (c.DC, c.K, c.ST, c.NTT, c.NST,
                                         c.NJC, c.SW, c.NSL)

    # ---------------- pools ----------------
    const = tc.alloc_tile_pool(name="const", bufs=1)
    wp = tc.alloc_tile_pool(name="wp", bufs=1)

    # ---------------- constants ----------------
    ident = const.tile([128, 128], f32)
    make_identity(nc, ident[:])
    ones_f = const.tile([128, 1], f32)
    nc.vector.memset(ones_f[:], 1.0)
    onesr_f = const.tile([1, 128], f32)
    nc.vector.memset(onesr_f[:], 1.0)
    ones_col = const.tile([128, 1], f32r)
    nc.vector.tensor_copy(out=ones_col[:], in_=ones_f[:])
    ones_row = const.tile([1, 128], f32r)
    nc.vector.tensor_copy(out=ones_row[:], in_=onesr_f[:])
    zpad = const.tile([128, 2], f32)
    nc.vector.memset(zpad[:], 0.0)
    identr = const.tile([128, 128], f32r)
    nc.vector.tensor_copy(out=identr[:], in_=ident[:])

    d_col = const.tile([128, NJC], f32)
    nc.sync.dma_start(out=d_col[:], in_=d_cm[:, :])
    d_rowr = const.tile([1, c.N // c.n_cores], f32r)

    def load_cols(dram_vec, name):
        cols = []
        for dc in range(DC):
            t = const.tile([128, 1], f32, name=f"{name}_{dc}")
            nc.sync.dma_start(out=t[:], in_=dram_vec[dc*128:(dc+1)*128, None])
            cols.append(t)
        return cols

    b1_col = load_cols(c1b, "b1")
    b2_col = load_cols(c2b, "b2")
    gb_col = load_cols(gb, "gb")
    hb_col = load_cols(hb, "hb") if has_c0 else None

    # conv weights -> f32r tiles [128(din), D(dout)] per (k, din_chunk)
    stage = tc.alloc_tile_pool(name="stage", bufs=3)
    d_row_f = stage.tile([1, c.N // c.n_cores], f32, name="d_row_f", bufs=1)
    nc.sync.dma_start(out=d_row_f[:], in_=d_rv[:, :])
    nc.vector.tensor_copy(out=d_rowr[:], in_=d_row_f[:])
    # gcn weights f32r [128(d), D(e)]
    gwr = []
    for dc in range(DC):
        st_ = stage.tile([128, c.D], f32, name="wstg3")
        nc.sync.dma_start(out=st_[:], in_=gwT[dc*128:(dc+1)*128, :])
        gr = wp.tile([128, c.D], f32r, name=f"gw_{dc}")
        nc.vector.tensor_copy(out=gr[:], in_=st_[:])
        gwr.append(gr)

    # hproj*g (host-folded) f32r [128(e), D(d)]
    hwgr = []
    for ec in range(DC):
        st_ = stage.tile([128, c.D], f32, name="wstg4")
        nc.sync.dma_start(out=st_[:], in_=hwg[ec*128:(ec+1)*128, :])
        hr = wp.tile([128, c.D], f32r, name=f"hwg_{ec}")
        nc.vector.tensor_copy(out=hr[:], in_=st_[:])
        hwgr.append(hr)
    stage.release()

    # ---------------- encode SBUF (persists all rows) ----------------
    epx = tc.alloc_tile_pool(name="epx", bufs=1)
    SP = c.S + 2  # row buffer width (2 zero pad cols at left)

    x_fm_rows = {}   # row -> list of DC tiles
    m_cols_rows = {}

    def gather_piece(row, tt, psum_pool, x_fm, m_cols):
        """Gather+transpose token tile tt (of 8) of `row` into x_fm."""
        row0 = row * c.S
        t0 = tt * 128
        if tt == 0:
            for dc in range(DC):
                nc.vector.tensor_copy(out=x_fm[dc][:, 0:2], in_=zpad[:, :])
        idx = epx.tile([128, 1], i32, name="idx", tag="idx", bufs=4)
        nc.sync.dma_start(out=idx[:], in_=x_in[row0+t0:row0+t0+128, None])
        mc = epx.tile([128, 1], f32, name="mc", tag="mc", bufs=16)
        nc.sync.dma_start(out=mc[:], in_=mask[row0+t0:row0+t0+128, None])
        m_cols.append(mc)
        x_tm = epx.tile([128, c.D], f32, name="x_tm", tag="x_tm", bufs=2)
        nc.gpsimd.indirect_dma_start(
            out=x_tm[:], out_offset=None, in_=embed[:],
            in_offset=bass.IndirectOffsetOnAxis(ap=idx[:, :1], axis=0))
        xm = epx.tile([128, c.D], f32r, name="xm", tag="xm", bufs=2)
        nc.vector.tensor_scalar_mul(xm[:], x_tm[:], mc[:])
        tp_ps = psum_pool.tile([128, c.D], f32r, name="tp_ps", tag="tp", bufs=1)
        for dc in range(DC):
            nc.tensor.transpose(out=tp_ps[:, dc*128:(dc+1)*128],
                                in_=xm[:, dc*128:(dc+1)*128],
                                identity=identr[:])
        for dc in range(DC):
            nc.vector.tensor_copy(out=x_fm[dc][:, 2+t0:2+t0+128],
                                  in_=tp_ps[:, dc*128:(dc+1)*128])

    def new_row_bufs(row):
        x_fm = [epx.tile([128, SP], f32r, name=f"xfm_{dcc}", tag=f"xfm{dcc}",
                         bufs=1) for dcc in range(DC)]
        x_fm_rows[row] = x_fm
        m_cols_rows[row] = []
        return x_fm, m_cols_rows[row]

    # ================= GCN phase (sharded over cores) =================
    # Each core computes its 256-row slice of H and of G, then AllGather.
    SG = c.N // c.n_cores  # 256: this core's i-slice width
    gtp = tc.alloc_tile_pool(name="gtp", bufs=1)
    G = [gtp.tile([128, c.N], f32r, name=f"G_{dc}") for dc in range(DC)]
    r0_row = gtp.tile([1, c.N], f32r, name="r0_row") if has_c0 else None

    gp = tc.alloc_tile_pool(name="gp", bufs=1)
    pgm = tc.alloc_tile_pool(name="pgm", bufs=1, space="PSUM")
    pg0 = tc.alloc_tile_pool(name="pg0", bufs=1, space="PSUM")
    dram = tc.alloc_tile_pool(name="dram", bufs=1, space="DRAM")
    NR0 = (c.D + 1) if has_c0 else c.D
    g_in = dram.tile([NR0, SG], f32, name="g_in")
    g_out = dram.tile([c.n_cores * NR0, SG], f32, name="g_out",
                      addr_space="Shared")

    # gather X = embed[sub_nodes]; fold d_j in the rounding copy
    Xr = []
    for jc in range(NJC):
        sidx = gp.tile([128, 1], i32, name="sidx", tag="sidx", bufs=4)
        nc.sync.dma_start(out=sidx[:], in_=sub_nodes[jc*128:(jc+1)*128, None])
        xstg = gp.tile([128, c.D], f32, name="xstg", tag="xstg", bufs=2)
        nc.gpsimd.indirect_dma_start(
            out=xstg[:], out_offset=None, in_=embed[:],
            in_offset=bass.IndirectOffsetOnAxis(ap=sidx[:, :1], axis=0))
        xt = gp.tile([128, c.D], f32r, name=f"Xg_{jc}", tag=f"Xr{jc}")
        nc.vector.tensor_scalar_mul(xt[:], xstg[:], d_col[:, jc:jc+1])
        Xr.append(xt)

    # m1 = X~^T @ A_hatT_loc (A_hat = A + I folded on host)
    m1_ps = [pgm.tile([128, SG], f32, name=f"m1p_{dc}", tag=f"gm{dc}",
                      bufs=1) for dc in range(DC)]
    for jc in range(NJC):
        a2 = gp.tile([128, SG], f32, name="a2", tag="a2", bufs=2)
        nc.sync.dma_start(out=a2[:], in_=A_T[jc*128:(jc+1)*128, :])
        ar2 = gp.tile([128, SG], f32r, name="ar2", tag="ar2", bufs=2)
        nc.vector.tensor_copy(out=ar2[:], in_=a2[:])
        for dc in range(DC):
            nc.tensor.matmul(out=m1_ps[dc][:],
                             lhsT=Xr[jc][:, dc*128:(dc+1)*128],
                             rhs=ar2[:], start=(jc == 0),
                             stop=(jc == NJC - 1))
    # d_i broadcast for this core's slice
    db_ps = pgm.tile([128, SG], f32, name="db_ps", tag="gsc", bufs=2)
    nc.tensor.matmul(out=db_ps[:], lhsT=ones_row[:], rhs=d_rowr[0:1, :],
                     start=True, stop=True)
    db_sb = gp.tile([128, SG], f32, name="db_sb", tag="db", bufs=2)
    nc.vector.tensor_copy(out=db_sb[:], in_=db_ps[:])
    m1s = []
    for dc in range(DC):
        m1t = gp.tile([128, SG], f32r, name=f"m1s_{dc}", tag=f"m1s{dc}",
                      bufs=1)
        nc.vector.tensor_mul(m1t[:], m1_ps[dc][:], db_sb[:])
        m1s.append(m1t)
    HTs = []
    for ec in range(DC):
        h2_ps = pgm.tile([128, SG], f32, name="h2_ps", tag="gsc", bufs=2)
        for dc in range(DC):
            nc.tensor.matmul(out=h2_ps[:],
                             lhsT=gwr[dc][:, ec*128:(ec+1)*128],
                             rhs=m1s[dc][:], start=(dc == 0),
                             stop=(dc == DC - 1))
        ht = gp.tile([128, SG], f32r, name=f"HT_{ec}", tag=f"ht{ec}",
                     bufs=2)
        nc.scalar.activation(out=ht[:], in_=h2_ps[:],
                             func=AF.Relu, bias=gb_col[ec][:])
        HTs.append(ht)
    # local G slice: G_loc[dc] = sum_ec hwgr[ec][:,dc] @ HTs[ec]
    for dc in range(DC):
        g_ps = pgm.tile([128, SG], f32, name="g_ps", tag="gsc", bufs=2)
        for ec in range(DC):
            nc.tensor.matmul(out=g_ps[:],
                             lhsT=hwgr[ec][:, dc*128:(dc+1)*128],
                             rhs=HTs[ec][:], start=(ec == 0),
                             stop=(ec == DC - 1))
        gl = gp.tile([128, SG], f32, name="gl", tag="gl", bufs=2)
        nc.vector.tensor_copy(out=gl[:], in_=g_ps[:])
        nc.sync.dma_start(out=g_in[dc*128:(dc+1)*128, :], in_=gl[:])
    if has_c0:
        r0_ps = pgm.tile([1, SG], f32, name="r0_ps", tag="gsc", bufs=2)
        for ec in range(DC):
            nc.tensor.matmul(out=r0_ps[0:1, :], lhsT=hb_col[ec][:],
                             rhs=HTs[ec][:], start=(ec == 0),
                             stop=(ec == DC - 1))
        r0l = gp.tile([1, SG], f32, name="r0l", tag="gl", bufs=2)
        nc.vector.tensor_copy(out=r0l[:], in_=r0_ps[:])
        nc.sync.dma_start(out=g_in[c.D:c.D+1, :], in_=r0l[:])

    nc.gpsimd.collective_compute(
        "AllGather", mybir.AluOpType.bypass,
        replica_groups=[list(range(c.n_cores))],
        ins=[g_in.opt()], outs=[g_out.opt()])

    # ---------------- row 0 gather (overlaps collective) ----------------
    x_fm0, m_cols0 = new_row_bufs(0)
    for tt in range(NTT * NST):
        gather_piece(0, tt, pg0, x_fm0, m_cols0)

    # ---- load gathered G into resident tiles ----
    for cc in range(c.n_cores):
        for dc in range(DC):
            nc.sync.dma_start(
                out=G[dc][:, cc*SG:(cc+1)*SG].bitcast(f32),
                in_=g_out[cc*NR0 + dc*128:cc*NR0 + (dc+1)*128, :])
        if has_c0:
            nc.sync.dma_start(out=r0_row[0:1, cc*SG:(cc+1)*SG].bitcast(f32),
                              in_=g_out[cc*NR0 + c.D:cc*NR0 + c.D + 1, :])
    gp.release()
    pg0.release()
    pgm.release()

    # conv weights (loaded after GCN so the A stream goes first)
    stage2 = tc.alloc_tile_pool(name="stage2", bufs=3)
    w1r, w2r = [], []
    for (wsrc, wdst) in ((c1w, w1r), (c2w, w2r)):
        for k in range(K):
            for dc in range(DC):
                st_ = stage2.tile([128, c.D], f32, name="wstg")
                nc.sync.dma_start(out=st_[:], in_=wsrc[k, dc*128:(dc+1)*128, :])
                wr = wp.tile([128, c.D], f32r, name=f"w_{len(wdst)}_{id(wdst)%97}")
                nc.vector.tensor_copy(out=wr[:], in_=st_[:])
                wdst.append(wr)
    stage2.release()

    # ================= encode + logits =================
    ep = tc.alloc_tile_pool(name="ep", bufs=1)
    pe = tc.alloc_tile_pool(name="pe", bufs=1, space="PSUM")

    for row in range(c.B_loc):
        x_fm = x_fm_rows[row]
        m_cols = m_cols_rows[row]
        row0 = row * c.S

        y1_fm = [ep.tile([128, SP], f32r, name=f"y1fm_{dcc}", tag=f"y1{dcc}",
                         bufs=1) for dcc in range(DC)]
        for dc in range(DC):
            nc.vector.tensor_copy(out=y1_fm[dc][:, 0:2], in_=zpad[:, :])

        # ---- conv1 both supertiles ----
        for dcout in range(DC):
            for st in range(NST):
                s0 = st * ST
                c1_ps = pe.tile([128, ST], f32, name="c1_ps", tag="c1", bufs=2)
                first = True
                for k in range(K):
                    for dci in range(DC):
                        nc.tensor.matmul(
                            out=c1_ps[:],
                            lhsT=w1r[k*DC+dci][:, dcout*128:(dcout+1)*128],
                            rhs=x_fm[dci][:, s0+k:s0+k+ST],
                            start=first, stop=(k == K-1 and dci == DC-1))
                        first = False
                nc.scalar.activation(out=y1_fm[dcout][:, 2+s0:2+s0+ST],
                                     in_=c1_ps[:], func=AF.Relu,
                                     bias=b1_col[dcout][:])

        # ---- conv2 + stats + LN cols, per supertile ----
        z_st, sc_st = [], []
        for st in range(NST):
            s0 = st * ST
            z, zq = [], []
            for dcout in range(DC):
                c2_ps = pe.tile([128, ST], f32, name="c2_ps", tag="c2", bufs=2)
                first = True
                for k in range(K):
                    for dci in range(DC):
                        nc.tensor.matmul(
                            out=c2_ps[:],
                            lhsT=w2r[k*DC+dci][:, dcout*128:(dcout+1)*128],
                            rhs=y1_fm[dci][:, s0+k:s0+k+ST],
                            start=first, stop=(k == K-1 and dci == DC-1))
                        first = False
                zt = ep.tile([128, ST], f32r, name=f"z_{dcout}",
                             tag=f"z{dcout}", bufs=1)
                nc.vector.scalar_tensor_tensor(
                    out=zt[:], in0=c2_ps[:], scalar=b2_col[dcout][:],
                    in1=x_fm[dcout][:, 2+s0:2+s0+ST].bitcast(f32),
                    op0=OP.add, op1=OP.add)
                z.append(zt)
                zqt = ep.tile([128, ST], f32r, name="zsq", tag="zsq", bufs=4)
                nc.scalar.square(zqt[:], zt[:].bitcast(f32))
                zq.append(zqt)
            st_ps = pe.tile([1, ST], f32, name="st_ps", tag="sa", bufs=1)
            for dc in range(DC):
                nc.tensor.matmul(out=st_ps[0:1, :], lhsT=ones_col[:],
                                 rhs=z[dc][:], start=(dc == 0),
                                 stop=(dc == DC-1))
            sq_ps = pe.tile([1, ST], f32, name="sq_ps", tag="c2", bufs=2)
            for dc in range(DC):
                nc.tensor.matmul(out=sq_ps[0:1, :], lhsT=ones_col[:],
                                 rhs=zq[dc][:], start=(dc == 0),
                                 stop=(dc == DC-1))
            mu_row = ep.tile([1, ST], f32, name="mu_row", tag="mu_row",
                             bufs=2)
            nc.scalar.mul(mu_row[:], st_ps[0:1, :], 1.0 / c.D)
            mu_rowr = ep.tile([1, ST], f32r, name="mu_rowr", tag="mu_rowr",
                              bufs=2)
            nc.vector.tensor_copy(out=mu_rowr[:], in_=mu_row[:])
            ms_row = ep.tile([1, ST], f32, name="ms_row", tag="ms_row",
                             bufs=2)
            nc.scalar.mul(ms_row[:], sq_ps[0:1, :], 1.0 / c.D)
            # transpose stats rows -> columns [128, NTT each]
            tr_ps = pe.tile([128, 2*NTT], f32, name="tr_ps", tag="c2",
                            bufs=2)
            for tt in range(NTT):
                nc.tensor.transpose(out=tr_ps[:, tt:tt+1],
                                    in_=mu_row[0:1, tt*128:(tt+1)*128],
                                    identity=ident[0:1, 0:1])
            for tt in range(NTT):
                nc.tensor.transpose(out=tr_ps[:, NTT+tt:NTT+tt+1],
                                    in_=ms_row[0:1, tt*128:(tt+1)*128],
                                    identity=ident[0:1, 0:1])
            # mu broadcast ([128, ST], K=1 matmul)
            mu_bc = pe.tile([128, ST], f32, name="mu_bc", tag="c2", bufs=2)
            nc.tensor.matmul(out=mu_bc[:], lhsT=ones_row[:], rhs=mu_rowr[:],
                             start=True, stop=True)
            # zc = z - mu
            zc = []
            for dc in range(DC):
                zct = ep.tile([128, ST], f32r, name=f"zc_{dc}", tag=f"zc{dc}",
                              bufs=2)
                nc.vector.scalar_tensor_tensor(
                    out=zct[:], in0=mu_bc[:], scalar=-1.0,
                    in1=z[dc][:].bitcast(f32), op0=OP.mult, op1=OP.add)
                zc.append(zct)
            # var/rstd in columns
            musq = ep.tile([128, NTT], f32, name="musq", tag="musq", bufs=2)
            nc.scalar.square(musq[:], tr_ps[:, 0:NTT])
            var_c = ep.tile([128, NTT], f32, name="var_c", tag="var_c", bufs=2)
            nc.vector.tensor_tensor(out=var_c[:], in0=tr_ps[:, NTT:2*NTT],
                                    in1=musq[:], op=OP.subtract)
            nc.vector.tensor_scalar_add(var_c[:], var_c[:], LN_EPS)
            nc.vector.reciprocal(var_c[:], var_c[:])
            rstd_c = ep.tile([128, NTT], f32, name="rstd_c", tag="rstd_c",
                             bufs=2)
            nc.scalar.sqrt(rstd_c[:], var_c[:])
            sc_cols = []
            for tt in range(NTT):
                sc = ep.tile([128, 1], f32, name="sc", tag="sc", bufs=16)
                nc.vector.tensor_scalar_mul(sc[:], rstd_c[:, tt:tt+1],
                                            m_cols[st*NTT+tt][:])
                sc_cols.append(sc)
            std_rows = None
            if has_c0:
                std_c = ep.tile([128, NTT], f32, name="std_c", tag="std_c",
                                bufs=2)
                nc.vector.tensor_tensor(out=std_c[:], in0=rstd_c[:],
                                        in1=var_c[:], op=OP.divide)
                std_rows = []
                for tt in range(NTT):
                    sr_ps = pe.tile([1, 128], f32, name="sr_ps", tag="sa",
                                    bufs=1)
                    nc.tensor.transpose(out=sr_ps[:],
                                        in_=std_c[:, tt:tt+1],
                                        identity=ident[0:1, 0:1])
                    sr = ep.tile([1, 128], f32r, name="sr", tag="sr", bufs=8)
                    nc.vector.tensor_copy(out=sr[:], in_=sr_ps[:])
                    std_rows.append(sr)
            z_st.append(zc)
            sc_st.append((sc_cols, std_rows))

        # ---- logits (+ next-row gather interleaved) ----
        nxt = None
        if row + 1 < c.B_loc:
            nxt = new_row_bufs(row + 1)
        piece = 0
        for st in range(NST):
            s0 = st * ST
            zc = z_st[st]
            sc_cols, std_rows = sc_st[st]
            for tt in range(NTT):
                for ns in range(NSL):
                    lg_ps = pe.tile([128, SW], f32, name="lg_ps", tag="lg",
                                    bufs=2)
                    for ec in range(DC):
                        nc.tensor.matmul(
                            out=lg_ps[:],
                            lhsT=zc[ec][:, tt*128:(tt+1)*128],
                            rhs=G[ec][:, ns*SW:(ns+1)*SW],
                            start=(ec == 0),
                            stop=(ec == DC-1 and not has_c0))
                        if has_c0 and ec == DC - 1:
                            nc.tensor.matmul(
                                out=lg_ps[:], lhsT=std_rows[tt][:],
                                rhs=r0_row[0:1, ns*SW:(ns+1)*SW],
                                start=False, stop=True)
                    lo = ep.tile([128, SW], f32, name="lo", tag="lo", bufs=4)
                    if ns % 2 == 0:
                        nc.scalar.mul(lo[:], lg_ps[:], sc_cols[tt][:])
                    else:
                        nc.vector.tensor_scalar_mul(lo[:], lg_ps[:],
                                                    sc_cols[tt][:])
                    t0g = row0 + s0 + tt * 128
                    eng = nc.scalar if ns % 2 == 0 else nc.sync
                    eng.dma_start(out=out[t0g:t0g+128, ns*SW:(ns+1)*SW],
                                  in_=lo[:])
                if nxt is not None:
                    gather_piece(row + 1, piece, pe, nxt[0], nxt[1])
                piece += 1
    pe.release()
    ep.release()
    gtp.release()
    epx.release()
    wp.release()
    const.release()


# ---------------------------------------------------------------------------
# host side
# ---------------------------------------------------------------------------

_CACHE = {}


def _get_program(cfg, has_c0=False):
    key = (cfg.V, cfg.D, cfg.B, cfg.S, cfg.N, cfg.K, cfg.n_cores, has_c0)
    if key not in _CACHE:
        _CACHE[key] = build_program(cfg, has_c0=has_c0)
    return _CACHE[key]


class _Runner:
    """Direct PJRT execution (no donation) so repeated runs are cheap."""

    def __init__(self, nc, n_cores):
        import jax
        from jax.sharding import Mesh, PartitionSpec, NamedSharding
        from jax.experimental.shard_map import shard_map
        from concourse import bass2jax
        bass2jax.install_neuronx_cc_hook()
        self.jax = jax
        self.n_cores = n_cores
        part_name = nc.partition_id_tensor.name if nc.partition_id_tensor else None
        in_names, out_names, out_avals, zero_outs = [], [], [], []
        for alloc in nc.m.functions[0].allocations:
            if not isinstance(alloc, mybir.MemoryLocationSet):
                continue
            name = alloc.memorylocations[0].name
            if alloc.kind == "ExternalInput":
                if name != part_name:
                    in_names.append(name)
            elif alloc.kind == "ExternalOutput":
                out_names.append(name)
                shape = tuple(alloc.tensor_shape)
                dtype = mybir.dt.np(alloc.dtype)
                out_avals.append(jax.core.ShapedArray(shape, dtype))
                zero_outs.append(np.zeros(shape, dtype))
        self.in_names, self.out_names = in_names, out_names
        self.out_avals, self.zero_outs = out_avals, zero_outs
        self.n_params = len(in_names)
        all_in = list(in_names) + list(out_names)
        if part_name:
            all_in.append(part_name)
        out_avals_t, all_in_t, out_names_t = (tuple(out_avals), tuple(all_in),
                                              tuple(out_names))

        def _body(*args):
            operands = list(args)
            if part_name:
                operands.append(bass2jax.partition_id_tensor())
            return tuple(bass2jax._bass_exec_p.bind(
                *operands, out_avals=out_avals_t, in_names=all_in_t,
                out_names=out_names_t, lowering_input_output_aliases=(),
                sim_require_finite=True, sim_require_nnan=True, nc=nc))

        devices = jax.devices()[:n_cores]
        self.mesh = Mesh(np.asarray(devices), ("core",))
        n_io = self.n_params + len(out_names)
        self.sharded = jax.jit(
            shard_map(_body, mesh=self.mesh,
                      in_specs=(PartitionSpec("core"),) * n_io,
                      out_specs=(PartitionSpec("core"),) * len(out_names),
                      check_rep=False),
            keep_unused=True)
        self.shard = NamedSharding(self.mesh, PartitionSpec("core"))

    def set_inputs(self, in_maps):
        jax = self.jax
        per_core = [[np.asarray(m[n]) for n in self.in_names] for m in in_maps]
        concat = [np.concatenate([per_core[cc][i] for cc in range(self.n_cores)],
                                 axis=0) for i in range(self.n_params)]
        concat += [np.zeros((self.n_cores * z.shape[0], *z.shape[1:]), z.dtype)
                   for z in self.zero_outs]
        self.dev_in = [jax.device_put(a, self.shard) for a in concat]
        jax.block_until_ready(self.dev_in)

    def run(self):
        outs = self.sharded(*self.dev_in)
        self.jax.block_until_ready(outs)
        return outs

    def run_np(self):
        outs = self.run()
        return [
            {n: np.asarray(outs[i]).reshape(self.n_cores,
                                            *self.out_avals[i].shape)[cc]
             for i, n in enumerate(self.out_names)}
            for cc in range(self.n_cores)
        ]


_RUNNER = {}


def make_in_maps(cfg, inputs):
    c = cfg
    x_in = np.asarray(inputs['x_in'])
    mask = np.asarray(inputs['mask_in']).astype(np.float32)
    A = np.asarray(inputs['A_sub']).astype(np.float32)
    A_hatT = A.T + np.eye(c.N, dtype=np.float32)
    ln_g = np.asarray(inputs['ln_g']).astype(np.float32)
    ln_b = np.asarray(inputs['ln_b']).astype(np.float32)
    hproj_w = np.asarray(inputs['hproj_w']).astype(np.float32)
    gcn_wT = np.ascontiguousarray(np.asarray(inputs['gcn_w']).T)
    # degree vector d = clip(rowsum(A)+1, 1e-6)^-0.5 (normalization prep)
    d = np.clip(A.sum(axis=1) + 1.0, 1e-6, None) ** -0.5
    d = d.astype(np.float32)
    d_col_mat = np.ascontiguousarray(d.reshape(c.NJC, 128).T)
    hproj_wg = np.ascontiguousarray(hproj_w * ln_g[None, :])
    hproj_b = np.ascontiguousarray(hproj_w @ ln_b)
    shared = {
        'sub_nodes': np.asarray(inputs['sub_nodes']).astype(np.int32),
        'embed': np.asarray(inputs['embed']).astype(np.float32),
        'conv1_w': np.asarray(inputs['conv1_w']).astype(np.float32),
        'conv1_b': np.asarray(inputs['conv1_b']).astype(np.float32),
        'conv2_w': np.asarray(inputs['conv2_w']).astype(np.float32),
        'conv2_b': np.asarray(inputs['conv2_b']).astype(np.float32),
        'gcn_wT': gcn_wT.astype(np.float32),
        'gcn_b': np.asarray(inputs['gcn_b']).astype(np.float32),
        'hproj_wg': hproj_wg,
        'hproj_b': hproj_b.astype(np.float32),
        'd_col_mat': d_col_mat,
    }
    in_maps = []
    SG = c.N // c.n_cores
    for cc in range(c.n_cores):
        rows = slice(cc * c.B_loc, (cc + 1) * c.B_loc)
        m = dict(shared)
        m['x_in_loc'] = np.ascontiguousarray(
            x_in[rows].reshape(-1)).astype(np.int32)
        m['mask_loc'] = np.ascontiguousarray(mask[rows].reshape(-1))
        m['A_subT'] = np.ascontiguousarray(A_hatT[:, cc*SG:(cc+1)*SG])
        m['d_row'] = np.ascontiguousarray(d[None, cc*SG:(cc+1)*SG])
        in_maps.append(m)
    return in_maps


def kernel(**inputs):
    cfg = Cfg()
    has_c0 = bool(np.any(np.asarray(inputs['ln_b']) != 0))
    nc = _get_program(cfg, has_c0)
    key = id(nc)
    if key not in _RUNNER:
        _RUNNER[key] = _Runner(nc, cfg.n_cores)
    r = _RUNNER[key]
    r.set_inputs(make_in_maps(cfg, inputs))
    res = r.run_np()
    out = np.concatenate(
        [res[cc]['logits_loc'].reshape(cfg.B_loc, cfg.S, cfg.N)
         for cc in range(cfg.n_cores)], axis=0)
    return out
